# revision 1
# baseline (speedup 1.0000x reference)
"""Trainium2 Bass kernel for the AnalyticalBoundedLineAttractor problem.

Reference semantics (per step, per sample):
    z = x @ W.T + b;  m = (z > 0);  A = diag(m) @ W - I;  c = m * b
    x_next = expm(A*dt) @ x + (expm(A*dt) - I) @ pinv(A) @ c

Key identity used here: A = G - I with G = diag(m) @ W, and -I commutes
with G, so expm(A*dt) = exp(-dt) * expm(G*dt).  Moreover the full affine
update is the top block of the augmented exponential

    [x_next; 1] = exp(-dt) * expm(dt * [[G, c'], [0, 1]]) @ [x; 1]

(with c' = m*b), which is an entire function of the matrix -- no pinv and
no full expm are needed.  A K-term Taylor series of matrix-VECTOR products
computes it exactly to fp32 precision (validated: rel err 9.3e-6 vs the
fp32 jax reference for K=3, 1.4e-6 for K=4):

    v_1 = relu(dt*z)
    v_k = m * ((dt/k) * W @ v_{k-1} + (dt^k/k!) * b)     k = 2..K
    x_next = exp(-dt) * (x + sum_k v_k)

Each matvec over the whole per-core batch is ONE (65x64)@(65x32) matmul on
the TensorEngine (batch on the free dim, D on partitions, the extra
partition row holds the ones that inject the bias term).

Sharding: data-parallel over batch, 256/8 = 32 samples per NeuronCore.
The 100-step trajectory stays resident in SBUF; one DMA out at the end.
"""

import math
import sys

import numpy as np

try:
    from concourse.bass_utils import run_bass_kernel_spmd
except ImportError:
    sys.path.insert(0, "/opt/trn_rl_repo")
    from concourse.bass_utils import run_bass_kernel_spmd

import concourse.bacc as bacc
import concourse.mybir as mybir
import concourse.tile as tile

DT = 0.05
T_STEPS = 100
DIM = 64
BATCH = 256
N_CORES = 8
BL = BATCH // N_CORES  # 32 samples per core
K = 3  # Taylor terms
LAM = math.exp(-DT)
F32 = mybir.dt.float32

_CACHE = {}


def _build_nc():
    nc = bacc.Bacc(None, target_bir_lowering=False)
    x0_ext = nc.declare_dram_parameter("x0", [DIM, BL], F32, isOutput=False)
    wts_ext = nc.declare_dram_parameter("wts", [DIM + 1, K * DIM], F32, isOutput=False)
    out_ext = nc.declare_dram_parameter("out", [DIM, T_STEPS * BL], F32, isOutput=True)

    AF = mybir.ActivationFunctionType
    OP = mybir.AluOpType

    with tile.TileContext(nc) as tc:
        with (
            tc.tile_pool(name="sb", bufs=1) as sb,
            tc.tile_pool(name="ps", bufs=4, space="PSUM") as ps,
        ):
            traj = sb.tile([DIM + 1, T_STEPS * BL], F32)
            V = sb.tile([DIM + 1, (K + 1) * BL], F32)
            wts = sb.tile([DIM + 1, K * DIM], F32)
            msk = sb.tile([DIM, BL], F32)

            nc.sync.dma_start(wts[:], wts_ext[:])
            nc.sync.dma_start(traj[0:DIM, 0:BL], x0_ext[:])
            nc.vector.memset(traj[DIM : DIM + 1, :], 1.0)
            nc.vector.memset(V[DIM : DIM + 1, :], 1.0)

            for t in range(T_STEPS - 1):
                x_slice = traj[:, t * BL : (t + 1) * BL]
                p0 = ps.tile([DIM, BL], F32)
                # p0 = dt*z = (dt*W) @ x + dt*b
                nc.tensor.matmul(p0[:], wts[:, 0:DIM], x_slice)
                # V slot 1 = lam * relu(p0) = lam * v_1
                nc.vector.tensor_scalar(
                    V[0:DIM, BL : 2 * BL], p0[:], 0.0, LAM, op0=OP.max, op1=OP.mult
                )
                # mask = (p0 > 0)   (same sign as z)
                nc.vector.tensor_scalar(msk[:], p0[:], 0.0, None, op0=OP.is_gt)
                # V slot 0 = lam * x_t   (ScalarE, off the critical path)
                nc.scalar.activation(
                    V[0:DIM, 0:BL], traj[0:DIM, t * BL : (t + 1) * BL], AF.Copy,
                    scale=LAM,
                )
                for k in range(2, K + 1):
                    pk = ps.tile([DIM, BL], F32)
                    # pk = (dt/k)*W @ v~_{k-1} + lam*(dt^k/k!)*b
                    nc.tensor.matmul(
                        pk[:],
                        wts[:, (k - 1) * DIM : k * DIM],
                        V[:, (k - 1) * BL : k * BL],
                    )
                    # V slot k = mask * pk
                    nc.vector.tensor_tensor(
                        V[0:DIM, k * BL : (k + 1) * BL], pk[:], msk[:], op=OP.mult
                    )
                # x_{t+1} = sum over the K+1 slots (= lam*(x_t + sum_k v_k))
                red_in = V[0:DIM, :].rearrange("p (s n) -> p n s", s=K + 1)
                nc.vector.tensor_reduce(
                    traj[0:DIM, (t + 1) * BL : (t + 2) * BL],
                    red_in,
                    axis=mybir.AxisListType.X,
                    op=OP.add,
                )

            nc.sync.dma_start(out_ext[:], traj[0:DIM, :])

    nc.compile()
    return nc


def _host_weights(W, b):
    """Stationary weight stack (DIM+1, K*DIM), fp64 math then fp32 cast."""
    W64 = W.astype(np.float64)
    b64 = b.astype(np.float64)
    wts = np.zeros((DIM + 1, K * DIM), np.float64)
    # k = 1: p0 = (dt*W) x + dt*b   -> lhsT rows 0..63 = dt*W.T, row 64 = dt*b
    wts[0:DIM, 0:DIM] = DT * W64.T
    wts[DIM, 0:DIM] = DT * b64
    for k in range(2, K + 1):
        a_k = DT**k / math.factorial(k)
        wts[0:DIM, (k - 1) * DIM : k * DIM] = (DT / k) * W64.T
        wts[DIM, (k - 1) * DIM : k * DIM] = LAM * a_k * b64
    return np.ascontiguousarray(wts.astype(np.float32))


def kernel(initial_position, W, b):
    x0 = np.asarray(initial_position, np.float32)
    W = np.asarray(W, np.float32)
    b = np.asarray(b, np.float32)

    if "nc" not in _CACHE:
        _CACHE["nc"] = _build_nc()
    nc = _CACHE["nc"]

    wts = _host_weights(W, b)
    in_maps = []
    for i in range(N_CORES):
        shard = np.ascontiguousarray(x0[i * BL : (i + 1) * BL].T)  # (DIM, BL)
        in_maps.append({"x0": shard, "wts": wts})

    res = run_bass_kernel_spmd(nc, in_maps, core_ids=list(range(N_CORES)))

    out = np.empty((BATCH, T_STEPS, DIM), np.float32)
    for i in range(N_CORES):
        core_out = res.results[i]["out"]  # (DIM, T_STEPS*BL)
        # (DIM, T, BL) -> (BL, T, DIM)
        out[i * BL : (i + 1) * BL] = core_out.reshape(DIM, T_STEPS, BL).transpose(
            2, 1, 0
        )
    return out


# revision 5
# speedup vs baseline: 1.8330x; 1.8330x over previous
"""Trainium2 Bass kernel for the AnalyticalBoundedLineAttractor problem.

Reference semantics (per step, per sample):
    z = x @ W.T + b;  m = (z > 0);  A = diag(m) @ W - I;  c = m * b
    x_next = expm(A*dt) @ x + (expm(A*dt) - I) @ pinv(A) @ c

Key identities used here:
  * A = G - I with G = diag(m) @ W, and -I commutes with G, so
    expm(A*dt) = exp(-dt) * expm(G*dt).
  * The affine update is the top block of an augmented matrix exponential
        [x_next; 1] = exp(-dt) * expm(dt*[[G, c'],[0, 1]]) @ [x; 1]
    which is an entire function -- no pinv, no expm needed.  A K-term
    Taylor series of matrix-VECTOR products evaluates it:
        v_1 = lam * relu(dt*z)                      (lam = exp(-dt))
        v_k = m * ((dt/k) * W @ v_{k-1} + lam*(dt^k/k!) * b)
        x_next = lam*x + sum_k v_k
    (each v_k here carries the lam factor; see the host weight prep).

K=2 with fp16 matmul inputs and fp16 state measures rel err 7.7e-4 vs
the fp32 jax reference (gate is 2e-2).  Each matvec over the whole
per-core batch is ONE (65x64)@(65x32) fp16 matmul (batch on the free
dim, D on partitions; partition row 64 holds ones, which injects the
bias via an extra weight row).

The 100-step fp16 trajectory tile IS the state: step t's final add
writes x_{t+1} straight into trajectory column block t+1, which is the
next step's matmul rhs.  One DMA out at the end; host casts to fp32.

Sharding: data-parallel over batch, 256/8 = 32 samples per NeuronCore.
"""

import math
import sys

import numpy as np

try:
    from concourse.bass_utils import run_bass_kernel_spmd
except ImportError:
    sys.path.insert(0, "/opt/trn_rl_repo")
    from concourse.bass_utils import run_bass_kernel_spmd

import concourse.bacc as bacc
import concourse.mybir as mybir
import concourse.tile as tile

DT = 0.05
T_STEPS = 100
DIM = 64
BATCH = 256
N_CORES = 8
BL = BATCH // N_CORES  # 32 samples per core
K = 2  # Taylor terms
LAM = math.exp(-DT)
F32 = mybir.dt.float32
F16 = mybir.dt.float16

_CACHE = {}


def _build_nc():
    nc = bacc.Bacc(None, target_bir_lowering=False)
    x0_ext = nc.declare_dram_parameter("x0h", [DIM + 1, BL], F16, isOutput=False)
    wts_ext = nc.declare_dram_parameter("wth", [DIM + 1, K * DIM], F16, isOutput=False)
    out_ext = nc.declare_dram_parameter("out", [DIM, T_STEPS * BL], F16, isOutput=True)

    AF = mybir.ActivationFunctionType
    OP = mybir.AluOpType

    with tile.TileContext(nc) as tc:
        with (
            tc.tile_pool(name="sb", bufs=1) as sb,
            tc.tile_pool(name="ps", bufs=2, space="PSUM") as ps,
        ):
            traj = sb.tile([DIM + 1, T_STEPS * BL], F16)
            V1 = sb.tile([DIM + 1, BL], F16)
            wts = sb.tile([DIM + 1, K * DIM], F16)
            msk = sb.tile([DIM, BL], F16)
            s1 = sb.tile([DIM, BL], F32)
            v2 = sb.tile([DIM, BL], F32)

            nc.sync.dma_start(wts[:], wts_ext[:])
            nc.vector.memset(traj[DIM : DIM + 1, :], 1.0)
            nc.vector.memset(V1[DIM : DIM + 1, :], 1.0)
            nc.sync.dma_start(traj[:, 0:BL], x0_ext[:])

            for t in range(T_STEPS - 1):
                x_in = traj[:, t * BL : (t + 1) * BL]  # [x_t; 1] fp16
                p0 = ps.tile([DIM, BL], F32)
                # p0 = dt*z = (dt*W) @ x + dt*b
                nc.tensor.matmul(p0[:], wts[:, 0:DIM], x_in)
                # V1 = lam * relu(p0) = v~_1   (fp16, feeds MM_2)
                nc.scalar.activation(V1[0:DIM, :], p0[:], AF.Relu, scale=LAM)
                # mask = (p0 > 0)
                nc.vector.tensor_scalar(msk[:], p0[:], 0.0, None, op0=OP.is_gt)
                # s1 = lam*x_t + v~_1   (off critical path)
                nc.vector.scalar_tensor_tensor(
                    s1[:], traj[0:DIM, t * BL : (t + 1) * BL], LAM, V1[0:DIM, :],
                    op0=OP.mult, op1=OP.add,
                )
                # p2 = (dt/2)*W @ v~_1 + lam*(dt^2/2)*b
                p2 = ps.tile([DIM, BL], F32)
                nc.tensor.matmul(p2[:], wts[:, DIM : 2 * DIM], V1[:])
                # x_{t+1} = s1 + mask*p2, via two DVE ops:
                nc.vector.tensor_tensor(v2[:], p2[:], msk[:], op=OP.mult)
                nc.vector.tensor_tensor(
                    traj[0:DIM, (t + 1) * BL : (t + 2) * BL], s1[:], v2[:], op=OP.add
                )

            nc.sync.dma_start(out_ext[:], traj[0:DIM, :])

    nc.compile()
    return nc


def _host_weights(W, b):
    """Stationary weight stack (DIM+1, K*DIM) fp16; fp64 math then cast."""
    W64 = W.astype(np.float64)
    b64 = b.astype(np.float64)
    wts = np.zeros((DIM + 1, K * DIM), np.float64)
    wts[0:DIM, 0:DIM] = DT * W64.T
    wts[DIM, 0:DIM] = DT * b64
    for k in range(2, K + 1):
        a_k = DT**k / math.factorial(k)
        wts[0:DIM, (k - 1) * DIM : k * DIM] = (DT / k) * W64.T
        wts[DIM, (k - 1) * DIM : k * DIM] = LAM * a_k * b64
    return np.ascontiguousarray(wts.astype(np.float16))


def _run_device(x0, W, b, **spmd_kwargs):
    if "nc" not in _CACHE:
        _CACHE["nc"] = _build_nc()
    nc = _CACHE["nc"]

    wts = _host_weights(W, b)
    in_maps = []
    for i in range(N_CORES):
        shard = np.empty((DIM + 1, BL), np.float16)
        shard[0:DIM] = x0[i * BL : (i + 1) * BL].T.astype(np.float16)
        shard[DIM] = 1.0
        in_maps.append({"x0h": shard, "wth": wts})

    return run_bass_kernel_spmd(
        nc, in_maps, core_ids=list(range(N_CORES)), **spmd_kwargs
    )


def kernel(initial_position, W, b):
    x0 = np.asarray(initial_position, np.float32)
    W = np.asarray(W, np.float32)
    b = np.asarray(b, np.float32)

    res = _run_device(x0, W, b)

    out = np.empty((BATCH, T_STEPS, DIM), np.float32)
    for i in range(N_CORES):
        core_out = res.results[i]["out"].astype(np.float32)  # (DIM, T*BL)
        out[i * BL : (i + 1) * BL] = core_out.reshape(DIM, T_STEPS, BL).transpose(
            2, 1, 0
        )
    return out
